# revision 9
# baseline (speedup 1.0000x reference)
"""Trainium2 Bass kernel for nn_MixedFeedFoward (DARTS-style mixed-architecture MLP).

Math: out = relu(x @ (m0*w0).T + bm0*b0) @ (m1*w1).T + bm1*b1
The DARTS masks are rank-structured.  With a = softmax(arch_embed),
b = softmax(arch_mlp), EMBED = (512,768,1024), RATIO = (2,3,4):

  s_e[h]     = sum_r b_r * [h < e*r]
  g_j[h]     = sum_{e_idx >= j} a_e * s_e[h]
  c_j        = sum_{e_idx >= j} a_e
  W0eff[h,d] = w0[h,d] * g_{blk(d)}[h]      blk(d): 0 for d<512, 1 for d<768, else 2
  bm0[h]     = g_0[h]
  W1eff[d,h] = w1[d,h] * g_{blk(d)}[h]
  bm1[d]     = c_{blk(d)}

g_j is constant on 256-aligned h segments, so masking reduces to 51 runtime
scalars computed on device from the arch inputs via one tiny matmul against a
constant 0/1 selection table (e9 broadcast to 128 columns makes the matmul
output land on all 128 partitions directly).  The softmax normalizer Z
(col 51, all-ones) is NOT applied on the critical path: w0 is masked with the
UNNORMALIZED gu = Z*g (h comes out scaled by Z), and the w1 mask scalars are
premultiplied by 1/Z^2, so L1 partials land at true scale and the tail
finalize is a plain add+store.  relu commutes with the positive Z scaling, so
numerics match the normalized form to bf16 rounding.

Sharding: data-parallel over the 4096 tokens -> 512 tokens per core; every
core streams the full weights.  PE roofline: 512 matmuls x 216 ns = 110.6 us.
The schedule is supply-shaped: measured per-core in-bound DMA is ~300 GB/s
for the first ~4 MiB (8-core HBM contention at start) and ~400 GB/s after,
while real matmuls demand 290 GB/s — so the opening is data-limited and the
binding bound is  t(first 4 MiB) + (512-32)*216ns + tail  ~=  131 us.

  - q1 (sync HWDGE) carries, in exact k-step consumption order: arch row,
    x0 x1, w0(hg0,pk0), x2 x3, w0(pk1), x4 x5, w0(pk2), x6 x7, w0(pk3),
    w0(hg1), then the steady w1/w0 phase stream.  Output stores also go on
    q1's engine (sync) — it is idle in the tail, unlike Scalar whose queue
    serializes ~0.6 us per DMA trigger behind the w1-mask activations.
  - PE warmup: NW1 junk matmuls (from ~7.5 us, HAM clock ramps), the tiny
    mask matmul (needs exp(arch) ~9.4 us), NW2 junk, then real work ~14 us.
  - PACE_* junk matmuls are interleaved into the first h-groups' k-steps,
    sized to the measured supply curve, so the PE never accumulates enough
    idle density to re-trigger the HAM half-clock window (which costs 2x on
    the following ~7 us).
  - L1(pr0) masks pj0/pj1 on DVE (0.3 us latency) instead of Scalar (1.7 us)
    so the first L1 matmuls aren't gated on mask production.
Steady state (pr = 1..3): L0 k-major (4 concurrent PSUM chains per 512-row
h-group), L1 dt-major over 8 output chains; demand 290 GB/s < supply.
"""

import os

import numpy as np

import concourse.bass as bass
import concourse.mybir as mybir
from concourse import bacc
from concourse.bass_utils import run_bass_kernel_spmd
from concourse.tile import TileContext

N_CORES = 8
D = 1024          # embed dim
H = 4096          # expansion dim
T = 512           # tokens per core (4096 total / 8 cores)
P = 128
SEG = 256         # h-segment size on which g_j is constant
NSEG = H // SEG   # 16
EMBED = (512, 768, 1024)
RATIO = (2, 3, 4)

F32 = mybir.dt.float32
BF16 = mybir.dt.bfloat16
AF = mybir.ActivationFunctionType
ALU = mybir.AluOpType

NW1 = int(os.environ.get("BASS_NW1", "30"))     # junk MMs before the mask matmul
NW2 = int(os.environ.get("BASS_NW2", "8"))    # junk MMs after it, before real work
# pacing junk inside the supply-starved opening: {after_kstep: count}
PACE_HG0 = {1: int(os.environ.get("BASS_PH0A", "8")),
            3: int(os.environ.get("BASS_PH0B", "3")),
            5: int(os.environ.get("BASS_PH0C", "5"))}
PACE_HG1 = {3: int(os.environ.get("BASS_PH1A", "0"))}
FILL2 = int(os.environ.get("BASS_FILL2", "0"))


def _build_k2() -> np.ndarray:
    """Constant 0/1 selection table: G_flat[col] = sum_i E9[i] * K2[i, col]
    where E9[e*3+r] = exp(ae[e] + am[r]).
    cols 0..47: col = j*16 + seg -> [e_idx >= j] * [seg*SEG < e*r]
    cols 48..50: col = 48 + j   -> [e_idx >= j]   (since sum_r b_r = 1)
    col 51: all ones -> sum(E9), the softmax normalizer Z
    """
    k2 = np.zeros((9, 52), dtype=np.float32)
    for ie, e in enumerate(EMBED):
        for ir, r in enumerate(RATIO):
            i = ie * 3 + ir
            for j in range(3):
                if ie >= j:
                    for seg in range(NSEG):
                        if seg * SEG < e * r:
                            k2[i, j * 16 + seg] = 1.0
                    k2[i, 48 + j] = 1.0
            k2[i, 51] = 1.0
    return k2


_K2 = _build_k2()

# d-block of each 128-wide d-chunk (0..7): [0,512)->0, [512,768)->1, [768,1024)->2
_DBLK = [0, 0, 0, 0, 1, 1, 2, 2]


def _build_nc() -> bass.Bass:
    nc = bacc.Bacc("TRN2", target_bir_lowering=False, debug=False)

    xT_d = nc.dram_tensor("xT", [D, T], F32, kind="ExternalInput")
    w0T_d = nc.dram_tensor("w0T", [D, H], F32, kind="ExternalInput")
    w1T_d = nc.dram_tensor("w1T", [H, D], F32, kind="ExternalInput")
    b0r_d = nc.dram_tensor("b0r", [P, H // P], F32, kind="ExternalInput")
    b1r_d = nc.dram_tensor("b1r", [P, D // P], F32, kind="ExternalInput")
    # arch = [ae9 | am9 | K2] packed in one tensor: a single 216B-row DMA
    arch_d = nc.dram_tensor("arch", [9, 54], F32, kind="ExternalInput")
    out_d = nc.dram_tensor("outT", [D, T], F32, kind="ExternalOutput")

    with TileContext(nc) as tc:
        with (
            tc.tile_pool(name="const", bufs=1) as const,
            tc.tile_pool(name="w0f", bufs=8) as w0f_pool,
            tc.tile_pool(name="xfp", bufs=1) as xf_pool,
            tc.tile_pool(name="w0p", bufs=16) as w0_pool,
            tc.tile_pool(name="w1f", bufs=7) as w1f_pool,
            tc.tile_pool(name="w1p", bufs=6) as w1_pool,
            tc.tile_pool(name="ps0", bufs=4, space="PSUM") as ps0_pool,
            tc.tile_pool(name="ps1", bufs=4, space="PSUM") as ps1_pool,
        ):
            # ---------------- tiny constants (no deps; DVE runs them first) --
            ones9 = const.tile([9, P], F32, tag="ones9")
            nc.vector.memset(ones9[:], 1.0)
            junk_w = const.tile([P, 2 * P], BF16, tag="junk_w")
            nc.gpsimd.memset(junk_w[:], 0.0)
            junk_x = const.tile([P, T], BF16, tag="junk_x")
            nc.gpsimd.memset(junk_x[:], 0.0)
            # PE warmup starts as early as possible (HAM busy-window)
            ps_w = ps0_pool.tile([P, T], F32, tag="ps0", name="ps_w")
            for i in range(NW1):
                sl = (i % 2) * P
                nc.tensor.matmul(
                    ps_w[:], junk_w[:, sl:sl + P], junk_x[:],
                    start=(i == 0), stop=(i == NW1 - 1),
                )

            # ---------------- arch-weight prep (q1 first: tiny, low latency) -
            arch_sb = const.tile([9, 54], F32, tag="arch_sb")
            nc.sync.dma_start(arch_sb[:], arch_d[:, :])
            k2_sb = arch_sb[:, 2:54]

            # e9 = exp(ae + am) in one ACT op; broadcast along free dim on DVE
            e9 = const.tile([9, 1], F32, tag="e9")
            nc.scalar.activation(e9[:], arch_sb[:, 0:1], AF.Exp, bias=arch_sb[:, 1:2])
            e9r = const.tile([9, P], F32, tag="e9r")
            nc.vector.tensor_scalar(e9r[:], ones9[:], e9[:, 0:1], None, ALU.mult)

            # biases ride the scalar HWDGE queue (triggers issue before exp)
            b0_sb = const.tile([P, H // P], F32, tag="b0_sb")
            nc.scalar.dma_start(b0_sb[:], b0r_d[:, :])
            b1_sb = const.tile([P, D // P], F32, tag="b1_sb")
            nc.scalar.dma_start(b1_sb[:], b1r_d[:, :])

            # ---------------- x + w0(hg0/hg1) on q1 in consumption order -----
            xfs = []
            xt_sb = []
            for k in range(D // P):
                xf = xf_pool.tile([P, T], F32, tag=f"xf{k}", name=f"xf{k}", bufs=1)
                t = const.tile([P, T], BF16, tag=f"xt{k}", name=f"xt{k}")
                xfs.append(xf)
                xt_sb.append(t)

            def load_x(k):
                nc.sync.dma_start(xfs[k][:], xT_d[k * P:(k + 1) * P, :])

            w0f_tiles = {}  # (hg, pk) -> tile

            def load_w0f(hg, pk):
                w0f = w0f_pool.tile([P, 1024], F32, tag="w0f", name="w0f")
                w0f_tiles[(hg, pk)] = w0f
                nc.sync.dma_start(
                    w0f[:].rearrange("p (k h) -> p k h", k=2),
                    w0T_d[
                        2 * pk * P:(2 * pk + 2) * P,
                        hg * 512:(hg + 1) * 512,
                    ].rearrange("(k p) h -> p k h", k=2),
                )

            # q1 order = k-step consumption order of L0(hg0)
            load_x(0)
            load_x(1)
            load_w0f(0, 0)
            load_x(2)
            load_x(3)
            load_w0f(0, 1)
            load_x(4)
            load_x(5)
            load_w0f(0, 2)
            load_x(6)
            load_x(7)
            load_w0f(0, 3)
            for pk in range(4):
                load_w0f(1, pk)

            # x casts: first two chunks on DVE (needed by the first k-steps),
            # the rest on Scalar (queued after exp)
            for k in range(2):
                nc.vector.tensor_copy(xt_sb[k][:], xfs[k][:])
            for k in range(2, D // P):
                nc.scalar.activation(xt_sb[k][:], xfs[k][:], AF.Copy)

            # ---------------- PE: mask matmul, then junk until data -------
            g_ps = ps1_pool.tile([P, T], F32, tag="ps1", name="g_ps")[:, 0:52]
            nc.tensor.matmul(g_ps[:], e9r[:], k2_sb[:], start=True, stop=True)
            ps_w2 = ps0_pool.tile([P, T], F32, tag="ps0", name="ps_w2")
            for i in range(NW2):
                sl = (i % 2) * P
                nc.tensor.matmul(
                    ps_w2[:], junk_w[:, sl:sl + P], junk_x[:],
                    start=(i == 0), stop=(i == NW2 - 1),
                )

            # gbu = unnormalized mask scalars [128, 52]; col 51 = Z
            gbu = const.tile([P, 52], F32, tag="gbu")
            nc.vector.tensor_copy(gbu[:], g_ps[:])

            # bb0[h] = gu_0[h] * b0[h]  (Z-scaled, matches Z-scaled h)
            bb0 = const.tile([P, H // P], F32, tag="bb0")
            nc.vector.tensor_tensor(
                bb0[:].rearrange("p (s i) -> p s i", i=2),
                b0_sb[:].rearrange("p (s i) -> p s i", i=2),
                gbu[:, 0:16].unsqueeze(2).to_broadcast((P, 16, 2)),
                ALU.mult,
            )

            # lazy: rec = 1/Z, gz = gbu/Z^2 (w1 mask scalars), bb1 = (c_j/Z)*b1
            rec = const.tile([P, 1], F32, tag="rec")
            nc.vector.reciprocal(rec[:], gbu[:, 51:52])
            rz2 = const.tile([P, 1], F32, tag="rz2")
            nc.vector.tensor_tensor(rz2[:], rec[:], rec[:], ALU.mult)
            gz = const.tile([P, 48], F32, tag="gz")
            nc.vector.tensor_scalar(gz[:], gbu[:, 0:48], rz2[:, 0:1], None, ALU.mult)
            bb1 = const.tile([P, D // P], F32, tag="bb1")
            for j, (c0, c1) in enumerate([(0, 4), (4, 6), (6, 8)]):
                nc.vector.tensor_scalar(
                    bb1[:, c0:c1], b1_sb[:, c0:c1],
                    gbu[:, 48 + j:49 + j], rec[:, 0:1], ALU.mult, ALU.mult,
                )

            # persistent hT and output accumulator
            ht_sb = [
                const.tile([P, T], BF16, tag=f"ht{m}", name=f"ht{m}")
                for m in range(H // P)
            ]
            outacc = [
                const.tile([P, T], F32, tag=f"oa{dt}", name=f"oa{dt}")
                for dt in range(D // P)
            ]

            def emit_fill(n, name, pool):
                if n <= 0:
                    return
                ps_f = pool.tile([P, T], F32, tag=pool is ps0_pool and "ps0" or "ps1", name=name)
                for i in range(n):
                    sl = (i % 2) * P
                    nc.tensor.matmul(
                        ps_f[:], junk_w[:, sl:sl + P], junk_x[:],
                        start=(i == 0), stop=(i == n - 1),
                    )

            def mask_w0(hg, pk):
                """mask+cast one w0f tile -> two [P, 512] bf16 chunks (DVE)."""
                w0f = w0f_tiles[(hg, pk)]
                chunks = []
                for c in range(2):
                    cbase = _DBLK[2 * pk + c] * 16 + hg * 2
                    w0m = w0_pool.tile([P, 512], BF16, tag="w0m", name="w0m")
                    nc.vector.tensor_tensor(
                        w0m[:].rearrange("p (s c) -> p s c", c=SEG),
                        w0f[:, c * 512:(c + 1) * 512].rearrange(
                            "p (s c) -> p s c", c=SEG
                        ),
                        gbu[:, cbase:cbase + 2]
                        .unsqueeze(2)
                        .to_broadcast((P, 2, SEG)),
                        ALU.mult,
                    )
                    chunks.append(w0m)
                return chunks

            pending_fin = []

            def emit_l0(pr, pace=None, preloaded=False):
                """L0 for h-groups 2pr, 2pr+1, k-major (4 live chains)."""
                for hg in (2 * pr, 2 * pr + 1):
                    pc = pace.get(hg, {}) if pace else {}
                    w0m_chunks = []
                    for pk in range(4):
                        if not preloaded:
                            load_w0f(hg, pk)
                        w0m_chunks.extend(mask_w0(hg, pk))
                        for _ in range(2):
                            if pending_fin:
                                pending_fin.pop(0)()
                    pool, ptag = (ps0_pool, "ps0") if hg % 2 == 0 else (ps1_pool, "ps1")
                    pss = [
                        pool.tile([P, T], F32, tag=ptag, name=f"ps0_{hg}_{ht}")
                        for ht in range(4)
                    ]
                    for k in range(D // P):
                        for ht in range(4):
                            nc.tensor.matmul(
                                pss[ht][:],
                                w0m_chunks[k][:, ht * P:(ht + 1) * P],
                                xt_sb[k][:],
                                start=(k == 0),
                                stop=(k == D // P - 1),
                            )
                        if k in pc:
                            emit_fill(pc[k], f"pace_{hg}_{k}", ps1_pool)
                    for ht in range(4):
                        m = hg * 4 + ht
                        nc.scalar.activation(
                            ht_sb[m][:], pss[ht][:], AF.Relu, bias=bb0[:, m:m + 1]
                        )

            def emit_l1(pr):
                """Layer 1 partial for h-group pair pr (K = 8 x 128)."""
                w1m_tiles = []
                for pj in range(4):
                    hc = pr * 8 + 2 * pj
                    w1f = w1f_pool.tile([P, 2048], F32, tag="w1f", name="w1f")
                    nc.sync.dma_start(
                        w1f[:].rearrange("p (k d) -> p k d", k=2),
                        w1T_d[hc * P:(hc + 2) * P, :].rearrange(
                            "(k p) d -> p k d", k=2
                        ),
                    )
                    seg_h = hc // 2
                    w1m = w1_pool.tile([P, 2048], BF16, tag="w1m", name="w1m")
                    ap3m = w1m[:].rearrange("p (k d) -> p k d", k=2)
                    ap3f = w1f[:].rearrange("p (k d) -> p k d", k=2)
                    # w1 masks run data-gated as tiles arrive: pj0/pj1 on
                    # DVE (low latency, L1-start critical), pj2/pj3 on the
                    # otherwise-idle GpSimd (ACT masks would queue behind the
                    # relu drains, holding w1f bufs and throttling prefetch)
                    for jd, (c0, c1) in enumerate([(0, 512), (512, 768), (768, 1024)]):
                        sc = gz[:, jd * 16 + seg_h:jd * 16 + seg_h + 1]
                        if pj <= 1:
                            nc.vector.tensor_scalar(
                                ap3m[:, :, c0:c1], ap3f[:, :, c0:c1],
                                sc, None, ALU.mult,
                            )
                        else:
                            nc.gpsimd.tensor_mul(
                                ap3m[:, :, c0:c1], ap3f[:, :, c0:c1],
                                sc.unsqueeze(2).to_broadcast((P, 1, c1 - c0)).to_broadcast((P, 2, c1 - c0)),
                            )
                    w1m_tiles.append(w1m)

                def finalize(dt, ps):
                    if pr == 0:
                        nc.scalar.activation(
                            outacc[dt][:], ps[:], AF.Identity, bias=bb1[:, dt:dt + 1]
                        )
                    elif pr < 3:
                        nc.vector.tensor_tensor(
                            outacc[dt][:], ps[:], outacc[dt][:], ALU.add
                        )
                    else:
                        # adds in halves (second overlaps the first), then one
                        # full-tile store on the idle sync queue
                        for c0, c1 in ((0, T // 2), (T // 2, T)):
                            nc.vector.tensor_tensor(
                                outacc[dt][:, c0:c1], ps[:, c0:c1],
                                outacc[dt][:, c0:c1], ALU.add,
                            )
                        nc.sync.dma_start(
                            out_d[dt * P:(dt + 1) * P, :], outacc[dt][:]
                        )

                if pr < 3:
                    # pj-major over all 8 output chains (all 8 PSUM banks):
                    # w1 pairs consumed in DMA arrival order, K-split deep
                    pss = [
                        (ps0_pool if dt < 4 else ps1_pool).tile(
                            [P, T], F32, tag="ps0" if dt < 4 else "ps1",
                            name=f"ps1_{pr}_{dt}"
                        )
                        for dt in range(8)
                    ]
                    for pj in range(4):
                        for j in (2 * pj, 2 * pj + 1):
                            for dt in range(8):
                                off = (j % 2) * 1024 + dt * P
                                nc.tensor.matmul(
                                    pss[dt][:],
                                    w1m_tiles[pj][:, off:off + P],
                                    ht_sb[pr * 8 + j][:],
                                    start=(j == 0),
                                    stop=(j == 7),
                                )
                    for dt in range(8):
                        pending_fin.append(
                            (lambda d=dt, p=pss[dt]: finalize(d, p))
                        )
                else:
                    # dt-major: chains end staggered so the finalize/store
                    # tail pipelines
                    for dt in range(D // P):
                        pool, ptag = (ps0_pool, "ps0") if dt < 4 else (ps1_pool, "ps1")
                        ps = pool.tile([P, T], F32, tag=ptag, name="ps1")
                        for j in range(8):
                            off = (j % 2) * 1024 + dt * P
                            nc.tensor.matmul(
                                ps[:],
                                w1m_tiles[j // 2][:, off:off + P],
                                ht_sb[pr * 8 + j][:],
                                start=(j == 0),
                                stop=(j == 7),
                            )
                        finalize(dt, ps)

            # ---------------- phases ----------------
            emit_l0(0, pace={0: PACE_HG0, 1: PACE_HG1}, preloaded=True)
            emit_fill(FILL2, "fill2", ps1_pool)
            emit_l1(0)
            for pr in range(1, 4):
                emit_l0(pr)
                emit_l1(pr)

    nc.compile()
    return nc


_NC_CACHE: dict[str, bass.Bass] = {}


def _get_nc() -> bass.Bass:
    key = f"{NW1}_{NW2}"
    if key not in _NC_CACHE:
        _NC_CACHE[key] = _build_nc()
    return _NC_CACHE[key]


def make_in_maps(x, w0, b0, w1, b1, arch_embed, arch_mlp):
    """Host-side layout prep (pure reshape/transpose/tile, no arithmetic)."""
    w0T = np.ascontiguousarray(w0.T)                       # [D, H]
    w1T = np.ascontiguousarray(w1.T)                       # [H, D]
    b0r = np.ascontiguousarray(b0.reshape(H // P, P).T)    # [P, 32]
    b1r = np.ascontiguousarray(b1.reshape(D // P, P).T)    # [P, 8]
    # packed [ae9 | am9 | K2]: pure repeat/tile/concat layout, no arithmetic
    arch = np.concatenate(
        [
            np.repeat(arch_embed, 3).reshape(9, 1),
            np.tile(arch_mlp, 3).reshape(9, 1),
            _K2,
        ],
        axis=1,
    ).astype(np.float32)
    arch = np.ascontiguousarray(arch)                      # [9, 54]
    x3 = x.reshape(N_CORES, T, D)
    return [
        {
            "xT": np.ascontiguousarray(x3[c].T),           # [D, T]
            "w0T": w0T,
            "w1T": w1T,
            "b0r": b0r,
            "b1r": b1r,
            "arch": arch,
        }
        for c in range(N_CORES)
    ]


def kernel(x, w0, b0, w1, b1, arch_embed, arch_mlp):
    x = np.asarray(x, dtype=np.float32)
    w0 = np.asarray(w0, dtype=np.float32)
    b0 = np.asarray(b0, dtype=np.float32)
    w1 = np.asarray(w1, dtype=np.float32)
    b1 = np.asarray(b1, dtype=np.float32)
    arch_embed = np.asarray(arch_embed, dtype=np.float32)
    arch_mlp = np.asarray(arch_mlp, dtype=np.float32)

    in_maps = make_in_maps(x, w0, b0, w1, b1, arch_embed, arch_mlp)
    nc = _get_nc()
    res = run_bass_kernel_spmd(nc, in_maps, core_ids=list(range(N_CORES)))
    out = np.stack([res.results[c]["outT"].T for c in range(N_CORES)], axis=0)
    return np.ascontiguousarray(out)  # [8, 512, 1024] float32


# revision 10
# speedup vs baseline: 1.0250x; 1.0250x over previous
"""Trainium2 Bass kernel for nn_MixedFeedFoward (DARTS-style mixed-architecture MLP).

Math: out = relu(x @ (m0*w0).T + bm0*b0) @ (m1*w1).T + bm1*b1
The DARTS masks are rank-structured.  With a = softmax(arch_embed),
b = softmax(arch_mlp), EMBED = (512,768,1024), RATIO = (2,3,4):

  s_e[h]     = sum_r b_r * [h < e*r]
  g_j[h]     = sum_{e_idx >= j} a_e * s_e[h]
  c_j        = sum_{e_idx >= j} a_e
  W0eff[h,d] = w0[h,d] * g_{blk(d)}[h]      blk(d): 0 for d<512, 1 for d<768, else 2
  bm0[h]     = g_0[h]
  W1eff[d,h] = w1[d,h] * g_{blk(d)}[h]
  bm1[d]     = c_{blk(d)}

g_j is constant on 256-aligned h segments, so masking reduces to 51 runtime
scalars computed on device from the arch inputs via one tiny matmul against a
constant 0/1 selection table (e9 broadcast to 128 columns makes the matmul
output land on all 128 partitions directly).  The softmax normalizer Z
(col 51, all-ones) is NOT applied on the critical path: w0 is masked with the
UNNORMALIZED gu = Z*g (h comes out scaled by Z), and the w1 mask scalars are
premultiplied by 1/Z^2, so L1 partials land at true scale and the tail
finalize is a plain add+store.  relu commutes with the positive Z scaling, so
numerics match the normalized form to bf16 rounding.

Sharding: data-parallel over the 4096 tokens -> 512 tokens per core; every
core streams the full weights.  PE roofline: 512 matmuls x 216 ns = 110.6 us.
The schedule is supply-shaped: measured per-core in-bound DMA is ~300 GB/s
for the first ~4 MiB (8-core HBM contention at start) and ~400 GB/s after,
while real matmuls demand 290 GB/s — so the opening is data-limited and the
binding bound is  t(first 4 MiB) + (512-32)*216ns + tail  ~=  131 us.

  - q1 (sync HWDGE) carries, in exact k-step consumption order: arch row,
    x0 x1, w0(hg0,pk0), x2 x3, w0(pk1), x4 x5, w0(pk2), x6 x7, w0(pk3),
    w0(hg1), then the steady w1/w0 phase stream.  Output stores also go on
    q1's engine (sync) — it is idle in the tail, unlike Scalar whose queue
    serializes ~0.6 us per DMA trigger behind the w1-mask activations.
  - PE warmup: NW1 junk matmuls (from ~7.5 us, HAM clock ramps), the tiny
    mask matmul (needs exp(arch) ~9.4 us), NW2 junk, then real work ~14 us.
  - PACE_* junk matmuls are interleaved into the first h-groups' k-steps,
    sized to the measured supply curve, so the PE never accumulates enough
    idle density to re-trigger the HAM half-clock window (which costs 2x on
    the following ~7 us).
  - L1(pr0) masks pj0/pj1 on DVE (0.3 us latency) instead of Scalar (1.7 us)
    so the first L1 matmuls aren't gated on mask production.
Steady state (pr = 1..3): L0 k-major (4 concurrent PSUM chains per 512-row
h-group), L1 dt-major over 8 output chains; demand 290 GB/s < supply.
"""

import os

import numpy as np

import concourse.bass as bass
import concourse.mybir as mybir
from concourse import bacc
from concourse.bass_utils import run_bass_kernel_spmd
from concourse.tile import TileContext

N_CORES = 8
D = 1024          # embed dim
H = 4096          # expansion dim
T = 512           # tokens per core (4096 total / 8 cores)
P = 128
SEG = 256         # h-segment size on which g_j is constant
NSEG = H // SEG   # 16
EMBED = (512, 768, 1024)
RATIO = (2, 3, 4)

F32 = mybir.dt.float32
BF16 = mybir.dt.bfloat16
AF = mybir.ActivationFunctionType
ALU = mybir.AluOpType

NW1 = int(os.environ.get("BASS_NW1", "28"))     # junk MMs before the mask matmul
NW2 = int(os.environ.get("BASS_NW2", "40"))    # junk MMs after it, before real work
# pacing junk inside the supply-starved opening: {after_kstep: count}
PACE_HG0 = {1: int(os.environ.get("BASS_PH0A", "8")),
            3: int(os.environ.get("BASS_PH0B", "3")),
            5: int(os.environ.get("BASS_PH0C", "5"))}
PACE_HG1 = {3: int(os.environ.get("BASS_PH1A", "0"))}
FILL2 = int(os.environ.get("BASS_FILL2", "0"))


def _build_k2() -> np.ndarray:
    """Constant 0/1 selection table: G_flat[col] = sum_i E9[i] * K2[i, col]
    where E9[e*3+r] = exp(ae[e] + am[r]).
    cols 0..47: col = j*16 + seg -> [e_idx >= j] * [seg*SEG < e*r]
    cols 48..50: col = 48 + j   -> [e_idx >= j]   (since sum_r b_r = 1)
    col 51: all ones -> sum(E9), the softmax normalizer Z
    """
    k2 = np.zeros((9, 52), dtype=np.float32)
    for ie, e in enumerate(EMBED):
        for ir, r in enumerate(RATIO):
            i = ie * 3 + ir
            for j in range(3):
                if ie >= j:
                    for seg in range(NSEG):
                        if seg * SEG < e * r:
                            k2[i, j * 16 + seg] = 1.0
                    k2[i, 48 + j] = 1.0
            k2[i, 51] = 1.0
    return k2


_K2 = _build_k2()

# d-block of each 128-wide d-chunk (0..7): [0,512)->0, [512,768)->1, [768,1024)->2
_DBLK = [0, 0, 0, 0, 1, 1, 2, 2]


def _build_nc() -> bass.Bass:
    nc = bacc.Bacc("TRN2", target_bir_lowering=False, debug=False)

    xT_d = nc.dram_tensor("xT", [D, T], F32, kind="ExternalInput")
    w0T_d = nc.dram_tensor("w0T", [D, H], F32, kind="ExternalInput")
    w1T_d = nc.dram_tensor("w1T", [H, D], F32, kind="ExternalInput")
    b0r_d = nc.dram_tensor("b0r", [P, H // P], F32, kind="ExternalInput")
    b1r_d = nc.dram_tensor("b1r", [P, D // P], F32, kind="ExternalInput")
    # arch = [ae9 | am9 | K2] packed in one tensor: a single 216B-row DMA
    arch_d = nc.dram_tensor("arch", [9, 54], F32, kind="ExternalInput")
    out_d = nc.dram_tensor("outT", [D, T], F32, kind="ExternalOutput")

    with TileContext(nc) as tc:
        with (
            tc.tile_pool(name="const", bufs=1) as const,
            tc.tile_pool(name="w0f", bufs=8) as w0f_pool,
            tc.tile_pool(name="xfp", bufs=1) as xf_pool,
            tc.tile_pool(name="w0p", bufs=16) as w0_pool,
            tc.tile_pool(name="w1f", bufs=7) as w1f_pool,
            tc.tile_pool(name="w1p", bufs=6) as w1_pool,
            tc.tile_pool(name="ps0", bufs=4, space="PSUM") as ps0_pool,
            tc.tile_pool(name="ps1", bufs=4, space="PSUM") as ps1_pool,
        ):
            # ---------------- tiny constants (no deps; DVE runs them first) --
            ones9 = const.tile([9, P], F32, tag="ones9")
            nc.vector.memset(ones9[:], 1.0)
            junk_w = const.tile([P, 2 * P], BF16, tag="junk_w")
            nc.gpsimd.memset(junk_w[:], 0.0)
            junk_x = const.tile([P, T], BF16, tag="junk_x")
            nc.gpsimd.memset(junk_x[:], 0.0)
            # PE warmup starts as early as possible (HAM busy-window)
            ps_w = ps0_pool.tile([P, T], F32, tag="ps0", name="ps_w")
            for i in range(NW1):
                sl = (i % 2) * P
                nc.tensor.matmul(
                    ps_w[:], junk_w[:, sl:sl + P], junk_x[:],
                    start=(i == 0), stop=(i == NW1 - 1),
                )

            # ---------------- arch-weight prep (q1 first: tiny, low latency) -
            arch_sb = const.tile([9, 54], F32, tag="arch_sb")
            nc.sync.dma_start(arch_sb[:], arch_d[:, :])
            k2_sb = arch_sb[:, 2:54]

            # e9 = exp(ae + am) in one ACT op; broadcast along free dim on DVE
            e9 = const.tile([9, 1], F32, tag="e9")
            nc.scalar.activation(e9[:], arch_sb[:, 0:1], AF.Exp, bias=arch_sb[:, 1:2])
            e9r = const.tile([9, P], F32, tag="e9r")
            nc.vector.tensor_scalar(e9r[:], ones9[:], e9[:, 0:1], None, ALU.mult)

            # biases ride the scalar HWDGE queue (triggers issue before exp)
            b0_sb = const.tile([P, H // P], F32, tag="b0_sb")
            nc.scalar.dma_start(b0_sb[:], b0r_d[:, :])
            b1_sb = const.tile([P, D // P], F32, tag="b1_sb")
            nc.scalar.dma_start(b1_sb[:], b1r_d[:, :])

            # ---------------- x + w0(hg0/hg1) on q1 in consumption order -----
            xfs = []
            xt_sb = []
            for k in range(D // P):
                xf = xf_pool.tile([P, T], F32, tag=f"xf{k}", name=f"xf{k}", bufs=1)
                t = const.tile([P, T], BF16, tag=f"xt{k}", name=f"xt{k}")
                xfs.append(xf)
                xt_sb.append(t)

            def load_x(k):
                nc.sync.dma_start(xfs[k][:], xT_d[k * P:(k + 1) * P, :])

            w0f_tiles = {}  # (hg, pk) -> tile

            def load_w0f(hg, pk):
                w0f = w0f_pool.tile([P, 1024], F32, tag="w0f", name="w0f")
                w0f_tiles[(hg, pk)] = w0f
                nc.sync.dma_start(
                    w0f[:].rearrange("p (k h) -> p k h", k=2),
                    w0T_d[
                        2 * pk * P:(2 * pk + 2) * P,
                        hg * 512:(hg + 1) * 512,
                    ].rearrange("(k p) h -> p k h", k=2),
                )

            # q1 order = k-step consumption order of L0(hg0)
            load_x(0)
            load_x(1)
            load_w0f(0, 0)
            load_x(2)
            load_x(3)
            load_w0f(0, 1)
            load_x(4)
            load_x(5)
            load_w0f(0, 2)
            load_x(6)
            load_x(7)
            load_w0f(0, 3)
            for pk in range(4):
                load_w0f(1, pk)

            # x casts: first two chunks on DVE (needed by the first k-steps),
            # the rest on Scalar (queued after exp)
            for k in range(2):
                nc.vector.tensor_copy(xt_sb[k][:], xfs[k][:])
            for k in range(2, D // P):
                nc.scalar.activation(xt_sb[k][:], xfs[k][:], AF.Copy)

            # ---------------- PE: mask matmul, then junk until data -------
            g_ps = ps1_pool.tile([P, T], F32, tag="ps1", name="g_ps")[:, 0:52]
            nc.tensor.matmul(g_ps[:], e9r[:], k2_sb[:], start=True, stop=True)
            ps_w2 = ps0_pool.tile([P, T], F32, tag="ps0", name="ps_w2")
            for i in range(NW2):
                sl = (i % 2) * P
                nc.tensor.matmul(
                    ps_w2[:, 0:64], junk_w[:, sl:sl + P], junk_x[:, 0:64],
                    start=(i == 0), stop=(i == NW2 - 1),
                )

            # gbu = unnormalized mask scalars [128, 52]; col 51 = Z
            gbu = const.tile([P, 52], F32, tag="gbu")
            nc.vector.tensor_copy(gbu[:], g_ps[:])

            # bb0[h] = gu_0[h] * b0[h]  (Z-scaled, matches Z-scaled h)
            bb0 = const.tile([P, H // P], F32, tag="bb0")
            nc.vector.tensor_tensor(
                bb0[:].rearrange("p (s i) -> p s i", i=2),
                b0_sb[:].rearrange("p (s i) -> p s i", i=2),
                gbu[:, 0:16].unsqueeze(2).to_broadcast((P, 16, 2)),
                ALU.mult,
            )

            # lazy: rec = 1/Z, gz = gbu/Z^2 (w1 mask scalars), bb1 = (c_j/Z)*b1
            rec = const.tile([P, 1], F32, tag="rec")
            nc.vector.reciprocal(rec[:], gbu[:, 51:52])
            rz2 = const.tile([P, 1], F32, tag="rz2")
            nc.vector.tensor_tensor(rz2[:], rec[:], rec[:], ALU.mult)
            gz = const.tile([P, 48], F32, tag="gz")
            nc.vector.tensor_scalar(gz[:], gbu[:, 0:48], rz2[:, 0:1], None, ALU.mult)
            bb1 = const.tile([P, D // P], F32, tag="bb1")
            for j, (c0, c1) in enumerate([(0, 4), (4, 6), (6, 8)]):
                nc.vector.tensor_scalar(
                    bb1[:, c0:c1], b1_sb[:, c0:c1],
                    gbu[:, 48 + j:49 + j], rec[:, 0:1], ALU.mult, ALU.mult,
                )

            # persistent hT and output accumulator
            ht_sb = [
                const.tile([P, T], BF16, tag=f"ht{m}", name=f"ht{m}")
                for m in range(H // P)
            ]
            outacc = [
                const.tile([P, T], F32, tag=f"oa{dt}", name=f"oa{dt}")
                for dt in range(D // P)
            ]

            def emit_fill(n, name, pool):
                if n <= 0:
                    return
                ps_f = pool.tile([P, T], F32, tag=pool is ps0_pool and "ps0" or "ps1", name=name)
                for i in range(n):
                    sl = (i % 2) * P
                    nc.tensor.matmul(
                        ps_f[:], junk_w[:, sl:sl + P], junk_x[:],
                        start=(i == 0), stop=(i == n - 1),
                    )

            def mask_w0(hg, pk):
                """mask+cast one w0f tile -> two [P, 512] bf16 chunks (DVE)."""
                w0f = w0f_tiles[(hg, pk)]
                chunks = []
                for c in range(2):
                    cbase = _DBLK[2 * pk + c] * 16 + hg * 2
                    w0m = w0_pool.tile([P, 512], BF16, tag="w0m", name="w0m")
                    nc.vector.tensor_tensor(
                        w0m[:].rearrange("p (s c) -> p s c", c=SEG),
                        w0f[:, c * 512:(c + 1) * 512].rearrange(
                            "p (s c) -> p s c", c=SEG
                        ),
                        gbu[:, cbase:cbase + 2]
                        .unsqueeze(2)
                        .to_broadcast((P, 2, SEG)),
                        ALU.mult,
                    )
                    chunks.append(w0m)
                return chunks

            pending_fin = []

            def emit_l0(pr, pace=None, preloaded=False):
                """L0 for h-groups 2pr, 2pr+1, k-major (4 live chains)."""
                for hg in (2 * pr, 2 * pr + 1):
                    pc = pace.get(hg, {}) if pace else {}
                    w0m_chunks = []
                    for pk in range(4):
                        if not preloaded:
                            load_w0f(hg, pk)
                        w0m_chunks.extend(mask_w0(hg, pk))
                        for _ in range(2):
                            if pending_fin:
                                pending_fin.pop(0)()
                    pool, ptag = (ps0_pool, "ps0") if hg % 2 == 0 else (ps1_pool, "ps1")
                    pss = [
                        pool.tile([P, T], F32, tag=ptag, name=f"ps0_{hg}_{ht}")
                        for ht in range(4)
                    ]
                    for k in range(D // P):
                        for ht in range(4):
                            nc.tensor.matmul(
                                pss[ht][:],
                                w0m_chunks[k][:, ht * P:(ht + 1) * P],
                                xt_sb[k][:],
                                start=(k == 0),
                                stop=(k == D // P - 1),
                            )
                        if k in pc:
                            emit_fill(pc[k], f"pace_{hg}_{k}", ps1_pool)
                    for ht in range(4):
                        m = hg * 4 + ht
                        nc.scalar.activation(
                            ht_sb[m][:], pss[ht][:], AF.Relu, bias=bb0[:, m:m + 1]
                        )

            def emit_l1(pr):
                """Layer 1 partial for h-group pair pr (K = 8 x 128)."""
                w1m_tiles = []
                for pj in range(4):
                    hc = pr * 8 + 2 * pj
                    w1f = w1f_pool.tile([P, 2048], F32, tag="w1f", name="w1f")
                    nc.sync.dma_start(
                        w1f[:].rearrange("p (k d) -> p k d", k=2),
                        w1T_d[hc * P:(hc + 2) * P, :].rearrange(
                            "(k p) d -> p k d", k=2
                        ),
                    )
                    seg_h = hc // 2
                    w1m = w1_pool.tile([P, 2048], BF16, tag="w1m", name="w1m")
                    ap3m = w1m[:].rearrange("p (k d) -> p k d", k=2)
                    ap3f = w1f[:].rearrange("p (k d) -> p k d", k=2)
                    # all w1 masks on DVE: they run data-gated as tiles
                    # arrive (ACT masks queue behind relu drains and hold w1f
                    # bufs, throttling prefetch; GpSimd is 3x too slow)
                    for jd, (c0, c1) in enumerate([(0, 512), (512, 768), (768, 1024)]):
                        sc = gz[:, jd * 16 + seg_h:jd * 16 + seg_h + 1]
                        nc.vector.tensor_scalar(
                            ap3m[:, :, c0:c1], ap3f[:, :, c0:c1],
                            sc, None, ALU.mult,
                        )
                    w1m_tiles.append(w1m)

                def finalize(dt, ps):
                    if pr == 0:
                        nc.scalar.activation(
                            outacc[dt][:], ps[:], AF.Identity, bias=bb1[:, dt:dt + 1]
                        )
                    elif pr < 3:
                        nc.vector.tensor_tensor(
                            outacc[dt][:], ps[:], outacc[dt][:], ALU.add
                        )
                    else:
                        # adds in halves (second overlaps the first), then one
                        # full-tile store on the idle sync queue
                        for c0, c1 in ((0, T // 2), (T // 2, T)):
                            nc.vector.tensor_tensor(
                                outacc[dt][:, c0:c1], ps[:, c0:c1],
                                outacc[dt][:, c0:c1], ALU.add,
                            )
                        nc.sync.dma_start(
                            out_d[dt * P:(dt + 1) * P, :], outacc[dt][:]
                        )

                if pr < 3:
                    # pj-major over all 8 output chains (all 8 PSUM banks):
                    # w1 pairs consumed in DMA arrival order, K-split deep
                    pss = [
                        (ps0_pool if dt < 4 else ps1_pool).tile(
                            [P, T], F32, tag="ps0" if dt < 4 else "ps1",
                            name=f"ps1_{pr}_{dt}"
                        )
                        for dt in range(8)
                    ]
                    for pj in range(4):
                        for j in (2 * pj, 2 * pj + 1):
                            for dt in range(8):
                                off = (j % 2) * 1024 + dt * P
                                nc.tensor.matmul(
                                    pss[dt][:],
                                    w1m_tiles[pj][:, off:off + P],
                                    ht_sb[pr * 8 + j][:],
                                    start=(j == 0),
                                    stop=(j == 7),
                                )
                    for dt in range(8):
                        pending_fin.append(
                            (lambda d=dt, p=pss[dt]: finalize(d, p))
                        )
                else:
                    # dt-major: chains end staggered so the finalize/store
                    # tail pipelines
                    for dt in range(D // P):
                        pool, ptag = (ps0_pool, "ps0") if dt < 4 else (ps1_pool, "ps1")
                        ps = pool.tile([P, T], F32, tag=ptag, name="ps1")
                        for j in range(8):
                            off = (j % 2) * 1024 + dt * P
                            nc.tensor.matmul(
                                ps[:],
                                w1m_tiles[j // 2][:, off:off + P],
                                ht_sb[pr * 8 + j][:],
                                start=(j == 0),
                                stop=(j == 7),
                            )
                        finalize(dt, ps)

            # ---------------- phases ----------------
            emit_l0(0, pace={0: PACE_HG0, 1: PACE_HG1}, preloaded=True)
            emit_fill(FILL2, "fill2", ps1_pool)
            emit_l1(0)
            for pr in range(1, 4):
                emit_l0(pr)
                emit_l1(pr)

    nc.compile()
    return nc


_NC_CACHE: dict[str, bass.Bass] = {}


def _get_nc() -> bass.Bass:
    key = f"{NW1}_{NW2}"
    if key not in _NC_CACHE:
        _NC_CACHE[key] = _build_nc()
    return _NC_CACHE[key]


def make_in_maps(x, w0, b0, w1, b1, arch_embed, arch_mlp):
    """Host-side layout prep (pure reshape/transpose/tile, no arithmetic)."""
    w0T = np.ascontiguousarray(w0.T)                       # [D, H]
    w1T = np.ascontiguousarray(w1.T)                       # [H, D]
    b0r = np.ascontiguousarray(b0.reshape(H // P, P).T)    # [P, 32]
    b1r = np.ascontiguousarray(b1.reshape(D // P, P).T)    # [P, 8]
    # packed [ae9 | am9 | K2]: pure repeat/tile/concat layout, no arithmetic
    arch = np.concatenate(
        [
            np.repeat(arch_embed, 3).reshape(9, 1),
            np.tile(arch_mlp, 3).reshape(9, 1),
            _K2,
        ],
        axis=1,
    ).astype(np.float32)
    arch = np.ascontiguousarray(arch)                      # [9, 54]
    x3 = x.reshape(N_CORES, T, D)
    return [
        {
            "xT": np.ascontiguousarray(x3[c].T),           # [D, T]
            "w0T": w0T,
            "w1T": w1T,
            "b0r": b0r,
            "b1r": b1r,
            "arch": arch,
        }
        for c in range(N_CORES)
    ]


def kernel(x, w0, b0, w1, b1, arch_embed, arch_mlp):
    x = np.asarray(x, dtype=np.float32)
    w0 = np.asarray(w0, dtype=np.float32)
    b0 = np.asarray(b0, dtype=np.float32)
    w1 = np.asarray(w1, dtype=np.float32)
    b1 = np.asarray(b1, dtype=np.float32)
    arch_embed = np.asarray(arch_embed, dtype=np.float32)
    arch_mlp = np.asarray(arch_mlp, dtype=np.float32)

    in_maps = make_in_maps(x, w0, b0, w1, b1, arch_embed, arch_mlp)
    nc = _get_nc()
    res = run_bass_kernel_spmd(nc, in_maps, core_ids=list(range(N_CORES)))
    out = np.stack([res.results[c]["outT"].T for c in range(N_CORES)], axis=0)
    return np.ascontiguousarray(out)  # [8, 512, 1024] float32
